# revision 19
# baseline (speedup 1.0000x reference)
"""Trainium2 Bass kernel for nn_GRU: embed -> GRU(seq) -> vocab projection.

Sharding strategy (8 NeuronCores):
  - The GRU recurrence is inherently sequential over S=256 steps and its
    per-step cost is dominated by streaming U (512x1536) through the PE
    array, which is independent of batch sharding.  So the recurrence is
    REPLICATED on all 8 cores (no collectives at all).
  - The output projection (134 GFLOP, 524 MB output) is sharded by VOCAB:
    each core gets a host-sliced Wd/bd vocab shard [512, 4000] and writes
    its own [16, 256, 4000] f32 output slice; the host concatenates.

On-device layout (all "transposed": hidden/3H on partitions, batch on free):
  - tokens are processed s-major (token t = s*16 + b)
  - embT  [128, 2*4096]  bf16: col k*4096 + t       (emb.T, k = EMB chunk)
  - xwT   [128, 256*192] bf16: col s*192 + m*16 + b (xw.T + biases; m = g*4+j)
  - hsT   [128, 4*4112]  bf16: col j*4112 + (s+1)*16 + b (h.T archive; first
          16 cols of each j-chunk are zeros = the s=-1 initial state)
  - U_sb  [128, 4*1536]  bf16: col k*1536 + c  (U tiles for lhsT)
Gate math runs in f32 on [128, 64..128] tiles (DVE/ACT); matmuls are bf16
with f32 PSUM accumulation.  Validated numerically: rel err ~3.7e-3.
"""

import numpy as np
from contextlib import ExitStack

import concourse.bass as bass
import concourse.tile as tile
import concourse.mybir as mybir
from concourse.bass_utils import run_bass_kernel_spmd
from concourse.masks import make_identity

VOCAB, EMB, HID = 32000, 256, 512
B, S = 16, 256
NCORES = 8
VSH = VOCAB // NCORES      # 4000 vocab cols per core
T = B * S                  # 4096 tokens (s-major)
H3 = 3 * HID               # 1536
NT = T // 128              # 32 token tiles
KJ = HID // 128            # 4 hidden chunks
KE = EMB // 128            # 2 embedding chunks
M3 = H3 // 128             # 12 gate-row tiles (m = g*4 + j)
HS_STRIDE = (S + 1) * 16   # 4112 cols per j-chunk in hsT
XW_C = 192                 # per-step xwT cols (g*64 + j*16 + b)
NCH = VSH // 500           # 8 psum column chunks in projection

bf = mybir.dt.bfloat16
f32 = mybir.dt.float32
i32 = mybir.dt.int32
AF = mybir.ActivationFunctionType


def _build_body(ctx, tc, x_in, E_in, W_in, U_in, b_in, Wd_in, bd_in, out):
    nc = tc.nc

    persist = ctx.enter_context(tc.tile_pool(name="persist", bufs=1))
    hsT = persist.tile([128, KJ * HS_STRIDE], bf)
    U_sb = persist.tile([128, KJ * H3], bf)
    bias_zr = persist.tile([128, 8], f32)   # b0+b1 per z/r m-tile
    bias_h = persist.tile([128, 4], f32)    # b0 per h-gate m-tile
    addin_h = persist.tile([128, 64], f32)  # b1 h-gate, (j,b) layout
    ident = persist.tile([128, 128], bf)
    make_identity(nc, ident[:])

    # Long-lived staging pool for DMA-written f32 tiles.  Never released, so
    # DMA writes into it only ever carry a single (same-tag WAR) dependency —
    # the HWDGE DMA instruction format supports just one sync-wait.
    stage = ctx.enter_context(tc.tile_pool(name="stage", bufs=2))

    # ---- biases ----
    b01 = persist.tile([128, 2 * M3], f32)  # col = t*12 + m, row = p
    nc.sync.dma_start(
        out=b01[:].rearrange("p (t m) -> p t m", m=M3),
        in_=b_in[:, :].rearrange("t (m p) -> p t m", p=128),
    )
    b0 = b01[:, 0:M3]
    b1 = b01[:, M3 : 2 * M3]
    nc.vector.tensor_add(bias_zr[:], b0[:, 0:8], b1[:, 0:8])
    nc.vector.tensor_copy(bias_h[:], b0[:, 8:12])
    for j in range(KJ):
        nc.vector.tensor_copy(
            addin_h[:, j * 16 : (j + 1) * 16],
            b1[:, 8 + j : 9 + j].to_broadcast([128, 16]),
        )

    # all 4096 token ids in one DMA: element (p, i) = token i*128 + p
    x_all = persist.tile([128, NT], i32)
    nc.sync.dma_start(
        out=x_all[:],
        in_=x_in[:].rearrange("(i p) -> p i", p=128),
    )

    # ---- U load + cast ----
    for k in range(KJ):
        us = stage.tile([128, H3], f32, tag="stg")
        # memset-first: makes the memset (an engine op, multi-wait capable)
        # the newest writer so the 1-wait-limited DMA has a single dep
        nc.vector.memset(us[:], 0.0)
        nc.sync.dma_start(out=us[:], in_=U_in[k * 128 : (k + 1) * 128, :])
        nc.vector.tensor_copy(U_sb[:, k * H3 : (k + 1) * H3], us[:])

    # xwT spans phase 0b + the recurrence, then its space is reused by Wd.
    with tc.tile_pool(name="xwT", bufs=1) as xwT_pool:
        xwT = xwT_pool.tile([128, S * XW_C], bf)

        # ---- phase 0a: gather E rows (s-major tokens), transpose to embT ----
        with (
            tc.tile_pool(name="embT_pool", bufs=1) as embT_pool,
            tc.tile_pool(name="gather", bufs=3) as gpool,
            tc.tile_pool(name="tp_psum", bufs=4, space="PSUM") as tppool,
        ):
            embT = embT_pool.tile([128, KE * T], bf)
            for i in range(NT):
                ef = gpool.tile([128, EMB], f32, tag="ef")
                nc.gpsimd.indirect_dma_start(
                    out=ef[:],
                    out_offset=None,
                    in_=E_in[:, :],
                    in_offset=bass.IndirectOffsetOnAxis(ap=x_all[:, i : i + 1], axis=0),
                )
                ec = gpool.tile([128, EMB], bf, tag="ec")
                nc.vector.tensor_copy(ec[:], ef[:])
                for k in range(KE):
                    tp = tppool.tile([128, 128], bf)
                    nc.tensor.transpose(tp[:], ec[:, k * 128 : (k + 1) * 128], ident[:])
                    nc.vector.tensor_copy(
                        embT[:, k * T + i * 128 : k * T + (i + 1) * 128], tp[:]
                    )

            # ---- phase 0b: xwT = W.T @ embT (+bias), strided (s,m,b) evict ----
            with (
                tc.tile_pool(name="wsb", bufs=1) as wsbp,
                tc.tile_pool(name="xw_psum", bufs=4, space="PSUM") as xwps,
            ):
                W_sb = wsbp.tile([128, KE * H3], bf)
                for k in range(KE):
                    ws = stage.tile([128, H3], f32, tag="stg")
                    nc.vector.memset(ws[:], 0.0)
                    nc.sync.dma_start(out=ws[:], in_=W_in[k * 128 : (k + 1) * 128, :])
                    nc.vector.tensor_copy(W_sb[:, k * H3 : (k + 1) * H3], ws[:])
                xw_view = xwT[:].rearrange("p (s c) -> p s c", c=XW_C)
                for m in range(M3):
                    bias_ap = bias_zr[:, m : m + 1] if m < 8 else bias_h[:, m - 8 : m - 7]
                    for sc in range(8):  # 512-token chunks
                        ps = xwps.tile([128, 512], f32, tag="xwps")
                        for k in range(KE):
                            nc.tensor.matmul(
                                ps[:],
                                W_sb[:, k * H3 + m * 128 : k * H3 + (m + 1) * 128],
                                embT[:, k * T + sc * 512 : k * T + (sc + 1) * 512],
                                start=(k == 0),
                                stop=(k == KE - 1),
                            )
                        outv = xw_view[:, sc * 32 : (sc + 1) * 32, m * 16 : (m + 1) * 16]
                        psv = ps[:].rearrange("p (s b) -> p s b", b=16)
                        biasv = bias_ap.to_broadcast([128, 512]).rearrange(
                            "p (s b) -> p s b", b=16
                        )
                        nc.vector.tensor_add(outv, psv, biasv)

        # ---- recurrence ----
        for j in range(KJ):
            nc.vector.memset(hsT[:, j * HS_STRIDE : j * HS_STRIDE + 16], 0.0)
        rec_stack = ExitStack()
        rec_ps_pool = rec_stack.enter_context(
            tc.tile_pool(name="rec_psum", bufs=2, space="PSUM")
        )
        gate_pool = rec_stack.enter_context(tc.tile_pool(name="gates", bufs=2))
        hsv = hsT[:].rearrange("p (j c) -> p j c", j=KJ)

        h_prev = gate_pool.tile([128, 64], f32, tag="h")
        nc.vector.memset(h_prev[:], 0.0)

        for s in range(S):
            ps = rec_ps_pool.tile([128, XW_C], f32, tag="recps")
            # NB: k must be the inner loop — start=True clears has_written
            # bits for the WHOLE bank, so accumulation groups (one per
            # 16-col m-slice) must run back-to-back, not interleaved.
            for m in range(M3):
                for k in range(KJ):
                    rhs = hsT[:, k * HS_STRIDE + s * 16 : k * HS_STRIDE + (s + 1) * 16]
                    nc.tensor.matmul(
                        ps[:, m * 16 : (m + 1) * 16],
                        U_sb[:, k * H3 + m * 128 : k * H3 + (m + 1) * 128],
                        rhs,
                        start=(k == 0),
                        stop=(k == KJ - 1),
                    )
            xs = s * XW_C
            zr_in = gate_pool.tile([128, 128], f32, tag="zr_in")
            nc.vector.tensor_add(zr_in[:], ps[:, 0:128], xwT[:, xs : xs + 128])
            zr = gate_pool.tile([128, 128], f32, tag="zr")
            nc.scalar.activation(zr[:], zr_in[:], AF.Sigmoid)
            rech = gate_pool.tile([128, 64], f32, tag="rech")
            nc.vector.tensor_add(rech[:], ps[:, 128:192], addin_h[:])
            rrh = gate_pool.tile([128, 64], f32, tag="rrh")
            nc.vector.tensor_mul(rrh[:], rech[:], zr[:, 64:128])
            hh_in = gate_pool.tile([128, 64], f32, tag="hh_in")
            nc.vector.tensor_add(hh_in[:], rrh[:], xwT[:, xs + 128 : xs + 192])
            hh = gate_pool.tile([128, 64], f32, tag="hh")
            nc.scalar.activation(hh[:], hh_in[:], AF.Tanh)
            d = gate_pool.tile([128, 64], f32, tag="d")
            nc.vector.tensor_sub(d[:], h_prev[:], hh[:])
            zd = gate_pool.tile([128, 64], f32, tag="zd")
            nc.vector.tensor_mul(zd[:], zr[:, 0:64], d[:])
            h_new = gate_pool.tile([128, 64], f32, tag="h")
            nc.vector.tensor_add(h_new[:], hh[:], zd[:])
            nc.scalar.copy(
                hsv[:, :, (s + 1) * 16 : (s + 2) * 16],
                h_new[:].rearrange("p (j b) -> p j b", b=16),
            )
            h_prev = h_new
        rec_stack.close()

    # ---- projection: out[:, :, slice] = hs @ Wd + bd ----
    with (
        tc.tile_pool(name="wd_sb", bufs=1) as wdp,
        tc.tile_pool(name="wd_stage", bufs=2) as wdst,
        tc.tile_pool(name="bd_pool", bufs=1) as bdp,
        tc.tile_pool(name="proj_psum", bufs=4, space="PSUM") as pps_pool,
        tc.tile_pool(name="proj_out", bufs=4) as pout,
    ):
        Wd_sb = wdp.tile([128, KJ * VSH], bf)
        for k in range(KJ):
            wds = wdst.tile([128, VSH], f32, tag="wds")
            # memset-first: absorbs the multi-proc address-reuse deps on an
            # engine op so the following DMA write has a single wait
            nc.vector.memset(wds[:], 0.0)
            nc.sync.dma_start(out=wds[:], in_=Wd_in[k * 128 : (k + 1) * 128, :])
            nc.vector.tensor_copy(Wd_sb[:, k * VSH : (k + 1) * VSH], wds[:])
        bd_sb = bdp.tile([1, VSH], f32)
        nc.vector.memset(bd_sb[:], 0.0)
        nc.sync.dma_start(out=bd_sb[:], in_=bd_in[:])
        bd_bf = bdp.tile([1, VSH], bf)
        nc.vector.tensor_copy(bd_bf[:], bd_sb[:])
        ones_col = bdp.tile([1, 128], bf)
        nc.vector.memset(ones_col[:], 1.0)
        # replicate bd across all 128 partitions via ones.T @ bd (K=1 matmul)
        bd_rep = bdp.tile([128, VSH], f32)
        for ni in range(NCH):
            bps = pps_pool.tile([128, 500], f32, tag="pps")
            nc.tensor.matmul(
                bps[:], ones_col[:], bd_bf[:, ni * 500 : (ni + 1) * 500],
                start=True, stop=True,
            )
            nc.vector.tensor_copy(bd_rep[:, ni * 500 : (ni + 1) * 500], bps[:])

        outv = out[:, :, :].rearrange("b s v -> s b v")
        for mi in range(NT):
            for ni in range(NCH):
                ps = pps_pool.tile([128, 500], f32, tag="pps")
                for k in range(KJ):
                    nc.tensor.matmul(
                        ps[:],
                        hsT[:, k * HS_STRIDE + 16 + mi * 128 : k * HS_STRIDE + 16 + (mi + 1) * 128],
                        Wd_sb[:, k * VSH + ni * 500 : k * VSH + (ni + 1) * 500],
                        start=(k == 0),
                        stop=(k == KJ - 1),
                    )
                ot = pout.tile([128, 500], f32, tag="ot")
                nc.vector.tensor_add(ot[:], ps[:], bd_rep[:, ni * 500 : (ni + 1) * 500])
                dst = outv[mi * 8 : (mi + 1) * 8, :, ni * 500 : (ni + 1) * 500]
                nc.sync.dma_start(out=dst, in_=ot[:])


def _legalize_dma_waits(nc):
    """walrus lowers dynamic-queue DMAs to PSEUDO_DMA_DIRECT2D which carries at
    most ONE sync-wait, but Tile can attach several (e.g. slot-reuse WAR on a
    compute engine + cross-queue WAW).  Move the excess waits onto wait-only
    InstEventSemaphore instructions placed immediately before the DMA in the
    same engine's (in-order) stream — the engine then stalls before enqueueing
    the descriptor, which preserves the happens-before edges."""
    n_split = 0
    for f in nc.m.functions:
        for blk in f.blocks:
            out = []
            for inst in blk.instructions:
                si = getattr(inst, "sync_info", None)
                if (
                    si is not None
                    and len(si.on_wait) > 1
                    and not isinstance(inst, mybir.InstEventSemaphore)
                ):
                    for w in si.on_wait[1:]:
                        n_split += 1
                        out.append(
                            mybir.InstEventSemaphore(
                                name=f"{inst.name}-ws{n_split}",
                                engine=inst.engine,
                                sync_info=mybir.SyncInfo(on_wait=[w], on_update=[]),
                            )
                        )
                    inst.sync_info = mybir.SyncInfo(
                        on_wait=[si.on_wait[0]], on_update=list(si.on_update)
                    )
                out.append(inst)
            blk.instructions = out
    return n_split


def build_nc():
    nc = bass.Bass()
    x_in = nc.declare_dram_parameter("xt", [T], i32, isOutput=False)
    E_in = nc.declare_dram_parameter("E", [VOCAB, EMB], f32, isOutput=False)
    W_in = nc.declare_dram_parameter("W", [EMB, H3], f32, isOutput=False)
    U_in = nc.declare_dram_parameter("U", [HID, H3], f32, isOutput=False)
    b_in = nc.declare_dram_parameter("b", [2, H3], f32, isOutput=False)
    Wd_in = nc.declare_dram_parameter("Wd", [HID, VSH], f32, isOutput=False)
    bd_in = nc.declare_dram_parameter("bd", [VSH], f32, isOutput=False)
    out = nc.declare_dram_parameter("out", [B, S, VSH], f32, isOutput=True)
    with tile.TileContext(nc) as tc, ExitStack() as ctx:
        _build_body(ctx, tc, x_in, E_in, W_in, U_in, b_in, Wd_in, bd_in, out)
    _legalize_dma_waits(nc)
    return nc


_NC = None


def run(inputs, trace=False, **kwargs):
    global _NC
    if _NC is None:
        _NC = build_nc()
    x = np.asarray(inputs["x"], dtype=np.int32)
    E = np.asarray(inputs["E"], dtype=np.float32)
    W = np.asarray(inputs["W"], dtype=np.float32)
    U = np.asarray(inputs["U"], dtype=np.float32)
    b = np.asarray(inputs["b"], dtype=np.float32)
    Wd = np.asarray(inputs["Wd"], dtype=np.float32)
    bd = np.asarray(inputs["bd"], dtype=np.float32)
    xt = np.ascontiguousarray(x.T).reshape(-1)  # s-major token order
    in_maps = []
    for c in range(NCORES):
        in_maps.append(
            {
                "xt": xt,
                "E": E,
                "W": W,
                "U": U,
                "b": b,
                "Wd": np.ascontiguousarray(Wd[:, c * VSH : (c + 1) * VSH]),
                "bd": np.ascontiguousarray(bd[c * VSH : (c + 1) * VSH]),
            }
        )
    res = run_bass_kernel_spmd(
        _NC, in_maps, core_ids=list(range(NCORES)), trace=trace, **kwargs
    )
    out = np.concatenate([r["out"] for r in res.results], axis=2)
    return out, res


def kernel(x, E, W, U, b, Wd, bd):
    out, _ = run(dict(x=x, E=E, W=W, U=U, b=b, Wd=Wd, bd=bd))
    return out


# revision 25
# speedup vs baseline: 1.0009x; 1.0009x over previous
"""Trainium2 Bass kernel for nn_GRU: embed -> GRU(seq) -> vocab projection.

Sharding strategy (8 NeuronCores):
  - The GRU recurrence is inherently sequential over S=256 steps and its
    per-step cost is dominated by feeding U (512x1536) into the PE array,
    which is independent of batch sharding.  So the recurrence is
    REPLICATED on all 8 cores (no collectives at all).
  - The output projection (134 GFLOP, 524 MB output) is sharded by VOCAB:
    each core gets a host-sliced Wd/bd vocab shard [512, 4000] (host
    pre-cast to bf16 - pure dtype prep, all math stays on device) and
    writes its own [16, 256, 4000] f32 output slice; host concatenates.
  - Projection matmul groups are INTERLEAVED into the recurrence (one
    [128-token x 500-vocab] group per step, once its tokens are 2+ steps
    old) so they fill PE stalls during the serial gate computation instead
    of running as a separate tail phase.

On-device layout (all "transposed": hidden/3H on partitions, batch on free):
  - tokens are processed s-major (token t = s*16 + b)
  - embT  DRAM [128, 2*4096] bf16: col k*4096 + t   (emb.T, k = EMB chunk)
  - xwT   [128, 256*192] bf16: col s*192 + m*16 + b (xw.T + biases; m=g*4+j)
  - hsT   [128, 4*4112]  bf16: col j*4112 + (s+1)*16 + b (h.T archive; first
          16 cols of each j-chunk are zeros = the s=-1 initial state)
  - U_sb  [128, 4*1536]  bf16: col k*1536 + c  (U tiles for lhsT)
Gate math runs in f32 on [128, 64..128] tiles (DVE/ACT); matmuls are bf16
with f32 PSUM accumulation.  Validated numerically: rel err ~3.6e-3.
"""

import numpy as np
from contextlib import ExitStack

import ml_dtypes
import concourse.bass as bass
import concourse.tile as tile
import concourse.mybir as mybir
from concourse.bass_utils import run_bass_kernel_spmd
from concourse.masks import make_identity

VOCAB, EMB, HID = 32000, 256, 512
B, S = 16, 256
NCORES = 8
VSH = VOCAB // NCORES      # 4000 vocab cols per core
T = B * S                  # 4096 tokens (s-major)
H3 = 3 * HID               # 1536
NT = T // 128              # 32 token tiles
KJ = HID // 128            # 4 hidden chunks
KE = EMB // 128            # 2 embedding chunks
M3 = H3 // 128             # 12 gate-row tiles (m = g*4 + j)
HS_STRIDE = (S + 1) * 16   # 4112 cols per j-chunk in hsT
XW_C = 192                 # per-step xwT cols (g*64 + j*16 + b)
NCH = VSH // 500           # 8 psum column chunks in projection

bf = mybir.dt.bfloat16
f32 = mybir.dt.float32
i32 = mybir.dt.int32
AF = mybir.ActivationFunctionType


def _build_body(ctx, tc, x_in, E_in, W_in, U_in, b_in, Wd_in, bd_in, out):
    nc = tc.nc

    persist = ctx.enter_context(tc.tile_pool(name="persist", bufs=1))
    hsT = persist.tile([128, KJ * HS_STRIDE], bf)
    U_sb = persist.tile([128, KJ * H3], bf)
    bias_zr = persist.tile([128, 8], f32)   # b0+b1 per z/r m-tile
    bias_h = persist.tile([128, 4], f32)    # b0 per h-gate m-tile
    addin_h = persist.tile([128, 64], f32)  # b1 h-gate, (j,b) layout
    ident = persist.tile([128, 128], bf)
    make_identity(nc, ident[:])

    # Long-lived staging pool for DMA-written f32 tiles.  Never released, so
    # DMA writes into it only ever carry a single (same-tag WAR) dependency —
    # the HWDGE DMA instruction format supports just one sync-wait.
    stage = ctx.enter_context(tc.tile_pool(name="stage", bufs=1))

    # Wd/bd (host-cast bf16) + projection staging live for the whole kernel
    # since projection interleaves with the recurrence.
    wdp = ctx.enter_context(tc.tile_pool(name="wd_sb", bufs=1))
    pout = ctx.enter_context(tc.tile_pool(name="proj_out", bufs=3))
    proj_ps_pool = ctx.enter_context(
        tc.tile_pool(name="proj_psum", bufs=2, space="PSUM")
    )
    Wd_sb = wdp.tile([128, KJ * VSH], bf)
    for k in range(KJ):
        nc.sync.dma_start(
            out=Wd_sb[:, k * VSH : (k + 1) * VSH],
            in_=Wd_in[k * 128 : (k + 1) * 128, :],
        )
    bd_rep = wdp.tile([128, VSH], bf)

    # ---- biases ----
    b01 = persist.tile([128, 2 * M3], f32)  # col = t*12 + m, row = p
    nc.sync.dma_start(
        out=b01[:].rearrange("p (t m) -> p t m", m=M3),
        in_=b_in[:, :].rearrange("t (m p) -> p t m", p=128),
    )
    b0 = b01[:, 0:M3]
    b1 = b01[:, M3 : 2 * M3]
    nc.vector.tensor_add(bias_zr[:], b0[:, 0:8], b1[:, 0:8])
    nc.vector.tensor_copy(bias_h[:], b0[:, 8:12])
    for j in range(KJ):
        nc.vector.tensor_copy(
            addin_h[:, j * 16 : (j + 1) * 16],
            b1[:, 8 + j : 9 + j].to_broadcast([128, 16]),
        )

    # all 4096 token ids in one DMA: element (p, i) = token i*128 + p
    x_all = persist.tile([128, NT], i32)
    nc.sync.dma_start(out=x_all[:], in_=x_in[:].rearrange("(i p) -> p i", p=128))

    # ---- U load + cast ----
    for k in range(KJ):
        us = stage.tile([128, H3], f32, tag="stg")
        # memset-first: makes the memset (an engine op, multi-wait capable)
        # the newest writer so the 1-wait-limited DMA has a single dep
        nc.vector.memset(us[:], 0.0)
        nc.sync.dma_start(out=us[:], in_=U_in[k * 128 : (k + 1) * 128, :])
        nc.vector.tensor_copy(U_sb[:, k * H3 : (k + 1) * H3], us[:])

    # emb.T staging lives in DRAM (SBUF is tight with Wd resident)
    embT_d = nc.dram_tensor("embT_scratch", [128, KE * T], bf)

    # ---- phase 0a: gather E rows (s-major tokens), transpose, spill emb.T ----
    with (
        tc.tile_pool(name="gather", bufs=3) as gpool,
        tc.tile_pool(name="tp_psum", bufs=2, space="PSUM") as tppool,
    ):
        for i in range(NT):
            ef = gpool.tile([128, EMB], f32, tag="ef")
            nc.gpsimd.indirect_dma_start(
                out=ef[:],
                out_offset=None,
                in_=E_in[:, :],
                in_offset=bass.IndirectOffsetOnAxis(ap=x_all[:, i : i + 1], axis=0),
            )
            ec = gpool.tile([128, EMB], bf, tag="ec")
            nc.vector.tensor_copy(ec[:], ef[:])
            for k in range(KE):
                tp = tppool.tile([128, 128], f32)
                # transpose via a NORMAL matmul (ec_chunk.T @ I): PE transpose
                # mode is avoided deliberately (walrus quirks), same cost here
                nc.tensor.matmul(
                    tp[:], ec[:, k * 128 : (k + 1) * 128], ident[:],
                    start=True, stop=True,
                )
                ev = gpool.tile([128, 128], bf, tag="tpev")
                nc.vector.tensor_copy(ev[:], tp[:])
                nc.sync.dma_start(
                    out=embT_d[:, k * T + i * 128 : k * T + (i + 1) * 128],
                    in_=ev[:],
                )

    # xwT spans phase 0b + the recurrence
    with tc.tile_pool(name="xwT", bufs=1) as xwT_pool:
        xwT = xwT_pool.tile([128, S * XW_C], bf)

        # ---- phase 0b: xwT = W.T @ embT (+bias), strided (s,m,b) evict ----
        with (
            tc.tile_pool(name="wsb", bufs=1) as wsbp,
            tc.tile_pool(name="rhsstream", bufs=2) as rhsp,
            tc.tile_pool(name="xw_psum", bufs=4, space="PSUM") as xwps,
        ):
            W_sb = wsbp.tile([128, KE * H3], bf)
            for k in range(KE):
                ws = stage.tile([128, H3], f32, tag="stg")
                nc.vector.memset(ws[:], 0.0)
                nc.sync.dma_start(out=ws[:], in_=W_in[k * 128 : (k + 1) * 128, :])
                nc.vector.tensor_copy(W_sb[:, k * H3 : (k + 1) * H3], ws[:])

            # bd replication: bd (bf16) -> all 128 partitions via ones.T @ bd
            with tc.tile_pool(name="bdtmp", bufs=1) as bdp:
                half = VSH // 2
                ones_col = bdp.tile([1, 128], bf)
                nc.vector.memset(ones_col[:], 1.0)
                for hi in range(2):
                    bd_sb = bdp.tile([1, half], bf, tag="bdsb")
                    nc.vector.memset(bd_sb[:], 0.0)
                    nc.sync.dma_start(out=bd_sb[:], in_=bd_in[hi * half : (hi + 1) * half])
                    for ni in range(NCH // 2):
                        bps = xwps.tile([128, 500], f32, tag="xwps")
                        nc.tensor.matmul(
                            bps[:], ones_col[:], bd_sb[:, ni * 500 : (ni + 1) * 500],
                            start=True, stop=True,
                        )
                        nc.vector.tensor_copy(
                            bd_rep[:, hi * half + ni * 500 : hi * half + (ni + 1) * 500],
                            bps[:],
                        )

            xw_view = xwT[:].rearrange("p (s c) -> p s c", c=XW_C)
            for sc in range(8):  # 512-token chunks
                rhs_t = []
                for k in range(KE):
                    rt = rhsp.tile([128, 512], bf, tag=f"rhs{k}")
                    nc.vector.memset(rt[:], 0.0)
                    nc.sync.dma_start(
                        out=rt[:],
                        in_=embT_d[:, k * T + sc * 512 : k * T + (sc + 1) * 512],
                    )
                    rhs_t.append(rt)
                for m in range(M3):
                    bias_ap = bias_zr[:, m : m + 1] if m < 8 else bias_h[:, m - 8 : m - 7]
                    ps = xwps.tile([128, 512], f32, tag="xwps")
                    for k in range(KE):
                        nc.tensor.matmul(
                            ps[:],
                            W_sb[:, k * H3 + m * 128 : k * H3 + (m + 1) * 128],
                            rhs_t[k][:],
                            start=(k == 0),
                            stop=(k == KE - 1),
                        )
                    outv = xw_view[:, sc * 32 : (sc + 1) * 32, m * 16 : (m + 1) * 16]
                    psv = ps[:].rearrange("p (s b) -> p s b", b=16)
                    biasv = bias_ap.to_broadcast([128, 512]).rearrange(
                        "p (s b) -> p s b", b=16
                    )
                    nc.vector.tensor_add(outv, psv, biasv)

        # ---- recurrence with interleaved projection ----
        for j in range(KJ):
            nc.vector.memset(hsT[:, j * HS_STRIDE : j * HS_STRIDE + 16], 0.0)

        outv_d = out[:, :, :].rearrange("b s v -> s b v")

        def emit_proj_group(mi, ni):
            ps = proj_ps_pool.tile([128, 500], f32, tag="pps")
            for k in range(KJ):
                nc.tensor.matmul(
                    ps[:],
                    hsT[:, k * HS_STRIDE + 16 + mi * 128 : k * HS_STRIDE + 16 + (mi + 1) * 128],
                    Wd_sb[:, k * VSH + ni * 500 : k * VSH + (ni + 1) * 500],
                    start=(k == 0),
                    stop=(k == KJ - 1),
                )
            ot = pout.tile([128, 500], f32, tag="ot")
            nc.vector.tensor_add(ot[:], ps[:], bd_rep[:, ni * 500 : (ni + 1) * 500])
            dst = outv_d[mi * 8 : (mi + 1) * 8, :, ni * 500 : (ni + 1) * 500]
            nc.sync.dma_start(out=dst, in_=ot[:])

        # schedule: proj group (mi, ni) rides along step s = 16 + mi*8 + ni
        sched = {}
        for mi in range(NT):
            for ni in range(NCH):
                s = 16 + mi * 8 + ni
                if s < S:
                    sched.setdefault(s, []).append((mi, ni))

        rec_stack = ExitStack()
        rec_ps_pool = rec_stack.enter_context(
            tc.tile_pool(name="rec_psum", bufs=2, space="PSUM")
        )
        gate_pool = rec_stack.enter_context(tc.tile_pool(name="gates", bufs=2))
        hsv = hsT[:].rearrange("p (j c) -> p j c", j=KJ)

        h_prev = gate_pool.tile([128, 64], f32, tag="h")
        nc.vector.memset(h_prev[:], 0.0)

        for s in range(S):
            ps = rec_ps_pool.tile([128, XW_C], f32, tag="recps")
            # NB: k must be the inner loop — start=True clears has_written
            # bits for the WHOLE bank, so accumulation groups (one per
            # 16-col m-slice) must run back-to-back, not interleaved.
            for m in range(M3):
                for k in range(KJ):
                    rhs = hsT[:, k * HS_STRIDE + s * 16 : k * HS_STRIDE + (s + 1) * 16]
                    nc.tensor.matmul(
                        ps[:, m * 16 : (m + 1) * 16],
                        U_sb[:, k * H3 + m * 128 : k * H3 + (m + 1) * 128],
                        rhs,
                        start=(k == 0),
                        stop=(k == KJ - 1),
                    )
            xs = s * XW_C
            zr_in = gate_pool.tile([128, 128], f32, tag="zr_in")
            nc.vector.tensor_add(zr_in[:], ps[:, 0:128], xwT[:, xs : xs + 128])
            zr = gate_pool.tile([128, 128], f32, tag="zr")
            nc.scalar.activation(zr[:], zr_in[:], AF.Sigmoid)
            rech = gate_pool.tile([128, 64], f32, tag="rech")
            nc.vector.tensor_add(rech[:], ps[:, 128:192], addin_h[:])
            rrh = gate_pool.tile([128, 64], f32, tag="rrh")
            nc.vector.tensor_mul(rrh[:], rech[:], zr[:, 64:128])
            hh_in = gate_pool.tile([128, 64], f32, tag="hh_in")
            nc.vector.tensor_add(hh_in[:], rrh[:], xwT[:, xs + 128 : xs + 192])
            hh = gate_pool.tile([128, 64], f32, tag="hh")
            nc.scalar.activation(hh[:], hh_in[:], AF.Tanh)
            d = gate_pool.tile([128, 64], f32, tag="d")
            nc.vector.tensor_sub(d[:], h_prev[:], hh[:])
            zd = gate_pool.tile([128, 64], f32, tag="zd")
            nc.vector.tensor_mul(zd[:], zr[:, 0:64], d[:])
            h_new = gate_pool.tile([128, 64], f32, tag="h")
            nc.vector.tensor_add(h_new[:], hh[:], zd[:])
            nc.scalar.copy(
                hsv[:, :, (s + 1) * 16 : (s + 2) * 16],
                h_new[:].rearrange("p (j b) -> p j b", b=16),
            )
            h_prev = h_new
            for mi, ni in sched.get(s, ()):
                emit_proj_group(mi, ni)
        rec_stack.close()

    # remaining projection groups (token tiles produced too late in the loop)
    done = {g for gs in sched.values() for g in gs}
    for mi in range(NT):
        for ni in range(NCH):
            if (mi, ni) not in done:
                emit_proj_group(mi, ni)


def _legalize_dma_waits(nc):
    """walrus lowers dynamic-queue DMAs to PSEUDO_DMA_DIRECT2D which carries at
    most ONE sync-wait, but Tile can attach several (e.g. slot-reuse WAR on a
    compute engine + cross-queue WAW).  The same limit bites most engine
    instruction formats in this walrus build.  Move the excess waits onto
    wait-only InstEventSemaphore instructions placed immediately before the
    instruction in the same engine's (in-order) stream — the engine stalls
    there first, which preserves the happens-before edges."""
    n_split = 0
    for f in nc.m.functions:
        for blk in f.blocks:
            out = []
            for inst in blk.instructions:
                si = getattr(inst, "sync_info", None)
                if (
                    si is not None
                    and len(si.on_wait) > 1
                    and not isinstance(inst, mybir.InstEventSemaphore)
                ):
                    for w in si.on_wait[1:]:
                        n_split += 1
                        out.append(
                            mybir.InstEventSemaphore(
                                name=f"{inst.name}-ws{n_split}",
                                engine=inst.engine,
                                sync_info=mybir.SyncInfo(on_wait=[w], on_update=[]),
                            )
                        )
                    inst.sync_info = mybir.SyncInfo(
                        on_wait=[si.on_wait[0]], on_update=list(si.on_update)
                    )
                out.append(inst)
            blk.instructions = out
    return n_split


def build_nc():
    nc = bass.Bass()
    x_in = nc.declare_dram_parameter("xt", [T], i32, isOutput=False)
    E_in = nc.declare_dram_parameter("E", [VOCAB, EMB], f32, isOutput=False)
    W_in = nc.declare_dram_parameter("W", [EMB, H3], f32, isOutput=False)
    U_in = nc.declare_dram_parameter("U", [HID, H3], f32, isOutput=False)
    b_in = nc.declare_dram_parameter("b", [2, H3], f32, isOutput=False)
    Wd_in = nc.declare_dram_parameter("Wd", [HID, VSH], bf, isOutput=False)
    bd_in = nc.declare_dram_parameter("bd", [VSH], bf, isOutput=False)
    out = nc.declare_dram_parameter("out", [B, S, VSH], f32, isOutput=True)
    with tile.TileContext(nc) as tc, ExitStack() as ctx:
        _build_body(ctx, tc, x_in, E_in, W_in, U_in, b_in, Wd_in, bd_in, out)
    _legalize_dma_waits(nc)
    return nc


_NC = None


def run(inputs, trace=False, **kwargs):
    global _NC
    if _NC is None:
        _NC = build_nc()
    x = np.asarray(inputs["x"], dtype=np.int32)
    E = np.asarray(inputs["E"], dtype=np.float32)
    W = np.asarray(inputs["W"], dtype=np.float32)
    U = np.asarray(inputs["U"], dtype=np.float32)
    b = np.asarray(inputs["b"], dtype=np.float32)
    Wd = np.asarray(inputs["Wd"], dtype=np.float32)
    bd = np.asarray(inputs["bd"], dtype=np.float32)
    xt = np.ascontiguousarray(x.T).reshape(-1)  # s-major token order
    in_maps = []
    for c in range(NCORES):
        in_maps.append(
            {
                "xt": xt,
                "E": E,
                "W": W,
                "U": U,
                "b": b,
                # host-side dtype prep only (sharding + bf16 cast); the
                # matmul itself runs on device
                "Wd": np.ascontiguousarray(
                    Wd[:, c * VSH : (c + 1) * VSH]
                ).astype(ml_dtypes.bfloat16),
                "bd": bd[c * VSH : (c + 1) * VSH].astype(ml_dtypes.bfloat16),
            }
        )
    res = run_bass_kernel_spmd(
        _NC, in_maps, core_ids=list(range(NCORES)), trace=trace, **kwargs
    )
    out = np.concatenate([r["out"] for r in res.results], axis=2)
    return out, res


def kernel(x, E, W, U, b, Wd, bd):
    out, _ = run(dict(x=x, E=E, W=W, U=U, b=b, Wd=Wd, bd=bd))
    return out
